# revision 6
# baseline (speedup 1.0000x reference)
"""AreaSelfAttention Trainium2 kernel (8 NeuronCores, pure data parallel).

Reference computation (per full input):
  pad x [4,256,252,252] -> [4,256,256,256]; 1x1 convs q,k (256->32), v (256->256);
  8x8 windows; attn = softmax(q^T k) over j; out = v @ attn^T; unwindow;
  final = gamma * out + x  (crop back to 252x252).

Strategy:
  - Host: pad + permute x into window-major pixel order, shard over
    (batch, window-row) across 8 cores; weights pre-transposed/packed; all
    matmul operands in bf16 (output error is damped by gamma ~0.1 and the
    exact f32-vs-bf16 residual x, well within the 2e-2 gate).
  - Device (per core, 16 "wrows" of 2048 pixels = 32 windows each):
      qk conv  : psum[64,512] = [WqT|WkT]^T @ x  (+rank-1 bias), ACT-evac to bf16
      vT conv  : psum[128pix,256] = x_blk^T @ WvT, DVE-evac (+bv) to bf16 [., 257]
                 with col 256 = 1/gamma (rowsum carrier)
      sT       : per window [64,64] = k_win^T q_win into [128,512] psum (16 win)
      exp      : ACT Exp psum -> eT bf16
      PV       : out^T[i,0:256] + rowsum/gamma[i] = eT_win^T @ [vT|1/gamma]
      recip    : DVE 1/x -> gamma/rowsum;  ACT Copy(scale=recip) evac -> bf16
      transpose: PE matmul-with-identity back to [c, pix] (psum bf16)
      final    : DVE add(psum_t, x_bf16) -> f32, DMA out
"""

from contextlib import ExitStack

import numpy as np
import ml_dtypes

import bass_rust as br
import concourse.bass as bass
import concourse.tile as tile
from concourse import mybir
from concourse.bass_utils import run_bass_kernel_spmd

FP32 = mybir.dt.float32
BF16 = mybir.dt.bfloat16
AF = mybir.ActivationFunctionType

B, C, H, W = 4, 256, 252, 252
A = 8
PH = PW = 256
NH = NW = 32
CR = 32
NCORES = 8
G = 16          # wrows per core
PIX = 2048      # pixels per wrow (32 windows * 64)
NWIN = 32       # windows per wrow


def _split_wide_waits(nc, max_waits=2):
    """walrus on this toolchain rejects >4 sync waits per instruction; move
    excess waits onto preceding same-engine NoOps (equivalent semantics)."""
    n = 0
    for fn in nc.m.functions:
        for bb in fn.blocks:
            insts = list(bb.instructions)
            new, changed = [], False
            for inst in insts:
                si = inst.sync_info
                waits = list(si.on_wait) if si is not None else []
                if len(waits) > max_waits:
                    changed = True
                    chunks = [waits[i:i + max_waits]
                              for i in range(0, len(waits), max_waits)]
                    for ch in chunks[:-1]:
                        nop = br.InstNoOp(name=f"I-wsplit-{n}", ins=[], outs=[])
                        n += 1
                        nop.engine = inst.engine
                        nop.sync_info = br.SyncInfo(on_wait=ch, on_update=[])
                        new.append(nop)
                    inst.sync_info = br.SyncInfo(
                        on_wait=chunks[-1], on_update=list(si.on_update))
                new.append(inst)
            if changed:
                bb.instructions = new
    return n


def build_nc():
    nc = bass.Bass()
    x_d = nc.declare_dram_parameter("x", [C, G, PIX], BF16, isOutput=False)
    wqk_d = nc.declare_dram_parameter("wqk", [2, 128, 64], BF16, isOutput=False)
    wvt_d = nc.declare_dram_parameter("wvt", [2, 128, 256], BF16, isOutput=False)
    bqk_d = nc.declare_dram_parameter("bqk", [1, 64], BF16, isOutput=False)
    bv_d = nc.declare_dram_parameter("bv", [256], FP32, isOutput=False)
    ig_d = nc.declare_dram_parameter("igamma", [1], BF16, isOutput=False)
    id_d = nc.declare_dram_parameter("ident", [128, 128], BF16, isOutput=False)
    out_d = nc.declare_dram_parameter("out", [C, G, PIX], FP32, isOutput=True)

    with tile.TileContext(nc) as tc, ExitStack() as ctx:
        consts = ctx.enter_context(tc.tile_pool(name="consts", bufs=1))
        xbp = ctx.enter_context(tc.tile_pool(name="xbp", bufs=3))
        qkp_sb = ctx.enter_context(tc.tile_pool(name="qkp_sb", bufs=2))
        ep = ctx.enter_context(tc.tile_pool(name="ep", bufs=3))
        vp = ctx.enter_context(tc.tile_pool(name="vp", bufs=5))
        rcp = ctx.enter_context(tc.tile_pool(name="rcp", bufs=8))
        otp = ctx.enter_context(tc.tile_pool(name="otp", bufs=4))
        obp = ctx.enter_context(tc.tile_pool(name="obp", bufs=2))

        qk_ps = ctx.enter_context(tc.tile_pool(name="qk_ps", bufs=1, space="PSUM"))
        st_ps = ctx.enter_context(tc.tile_pool(name="st_ps", bufs=1, space="PSUM"))
        vt_ps = ctx.enter_context(tc.tile_pool(name="vt_ps", bufs=2, space="PSUM"))
        pv_ps = ctx.enter_context(tc.tile_pool(name="pv_ps", bufs=2, space="PSUM"))
        pt_ps = ctx.enter_context(tc.tile_pool(name="pt_ps", bufs=1, space="PSUM"))

        # ---- constants ----
        wqk_b = consts.tile([128, 2, 64], BF16, tag="wqk")
        for h in range(2):
            nc.sync.dma_start(out=wqk_b[:, h, :], in_=wqk_d[h])
        wvt_b = consts.tile([128, 2, 256], BF16, tag="wvt")
        for h in range(2):
            nc.sync.dma_start(out=wvt_b[:, h, :], in_=wvt_d[h])
        bqk_b = consts.tile([1, 64], BF16, tag="bqk")
        nc.sync.dma_start(out=bqk_b, in_=bqk_d[:])
        # bv replicated to 128 partitions x 2 copies (batched vT evacuation)
        bv2 = consts.tile([128, 2, 256], FP32, tag="bv2")
        bv_ap = bv_d[:]
        bv_bcast = bass.AP(tensor=bv_ap.tensor, offset=bv_ap.offset,
                           ap=[[0, 128], [0, 2]] + list(bv_ap.ap))
        nc.sync.dma_start(out=bv2, in_=bv_bcast)
        ig_b = consts.tile([128, 1], BF16, tag="ig")
        ig_ap = ig_d[:]
        ig_bcast = bass.AP(tensor=ig_ap.tensor, offset=ig_ap.offset,
                           ap=[[0, 128]] + list(ig_ap.ap))
        nc.sync.dma_start(out=ig_b, in_=ig_bcast)
        ident_b = consts.tile([128, 128], BF16, tag="ident")
        nc.sync.dma_start(out=ident_b, in_=id_d[:])
        ones_b = consts.tile([1, 512], BF16, tag="ones")
        nc.vector.memset(ones_b, 1.0)

        # ---- main loop over wrows ----
        for g in range(G):
            xb0 = xbp.tile([128, PIX], BF16, tag="xb0")
            nc.sync.dma_start(out=xb0, in_=x_d[0:128, g, :])
            xb1 = xbp.tile([128, PIX], BF16, tag="xb1")
            nc.sync.dma_start(out=xb1, in_=x_d[128:256, g, :])
            xbs = [xb0, xb1]

            # qk conv -> qk_sb [64, 2048] (rows 0:32 q, 32:64 k), bf16
            qk_sb = qkp_sb.tile([64, PIX], BF16, tag="qk")
            for blk in range(4):
                s = slice(blk * 512, (blk + 1) * 512)
                qps = qk_ps.tile([64, 512], FP32, tag="qkps")
                nc.tensor.matmul(qps, wqk_b[:, 0, :], xb0[:, s],
                                 start=True, stop=False)
                nc.tensor.matmul(qps, wqk_b[:, 1, :], xb1[:, s],
                                 start=False, stop=False)
                nc.tensor.matmul(qps, bqk_b, ones_b, start=False, stop=True)
                nc.scalar.activation(out=qk_sb[:, s], in_=qps, func=AF.Copy)

            # k at partition base 0 (sbuf->sbuf DMA partition shift)
            k0 = qkp_sb.tile([32, PIX], BF16, tag="k0")
            nc.sync.dma_start(out=k0, in_=qk_sb[32:64, :])

            # vT conv: per 2 pair-blocks (256 pixels) -> vt [128, 2, 257] bf16
            vt_tiles = []
            for vg in range(8):
                vps = vt_ps.tile([128, 2, 256], FP32, tag="vtps")
                for j in range(2):
                    p0 = vg * 256 + j * 128
                    nc.tensor.matmul(vps[:, j, :], xb0[:, p0:p0 + 128],
                                     wvt_b[:, 0, :], start=True, stop=False)
                    nc.tensor.matmul(vps[:, j, :], xb1[:, p0:p0 + 128],
                                     wvt_b[:, 1, :], start=False, stop=True)
                vt = vp.tile([128, 2, 257], BF16, tag="vt")
                nc.vector.tensor_add(vt[:, :, 0:256], vps, bv2)
                for j in range(2):
                    nc.gpsimd.tensor_copy(out=vt[:, j, 256:257], in_=ig_b)
                vt_tiles.append(vt)

            # sT per window into [128,512] psum (16 windows each), then exp
            eTs = []
            for sg in range(2):
                sps = st_ps.tile([128, 512], FP32, tag="stps")
                for wl in range(16):
                    w = sg * 16 + wl
                    half, cb = wl & 1, wl >> 1
                    ws = slice(w * 64, (w + 1) * 64)
                    nc.tensor.matmul(
                        sps[half * 64:(half + 1) * 64, cb * 64:(cb + 1) * 64],
                        k0[:, ws], qk_sb[0:32, ws], start=True, stop=True)
                eT = ep.tile([128, 512], BF16, tag="eT")
                nc.scalar.activation(out=eT, in_=sps, func=AF.Exp)
                eTs.append(eT)

            ob0 = obp.tile([128, PIX], FP32, tag="ob0")
            ob1 = obp.tile([128, PIX], FP32, tag="ob1")
            obs = [ob0, ob1]

            # PV + normalize + transpose + residual, 4 pair-blocks per group
            for q in range(4):
                pts = [pt_ps.tile([128, 512], BF16, tag=f"pt{h}",
                                  name=f"pt_{g}_{q}_{h}")
                       for h in range(2)]
                for t in range(4):
                    p = q * 4 + t          # pair-block index (128 pixels)
                    sg, m = p // 8, p % 8
                    eT = eTs[sg]
                    vt = vt_tiles[p // 2]
                    j = p % 2
                    ms = slice(m * 64, (m + 1) * 64)
                    pv = pv_ps.tile([128, 257], FP32, tag="pv")
                    nc.tensor.matmul(pv[0:64, :], eT[0:64, ms],
                                     vt[0:64, j, :], start=True, stop=True)
                    nc.tensor.matmul(pv[64:128, :], eT[64:128, ms],
                                     vt[64:128, j, :], start=True, stop=True)
                    rc = rcp.tile([128, 1], FP32, tag="rc")
                    nc.vector.reciprocal(out=rc, in_=pv[:, 256:257])
                    ot = otp.tile([128, 256], BF16, tag="ot")
                    nc.scalar.activation(out=ot, in_=pv[:, 0:256],
                                         func=AF.Copy, scale=rc)
                    for h in range(2):
                        nc.tensor.transpose(
                            pts[h][:, t * 128:(t + 1) * 128],
                            ot[:, h * 128:(h + 1) * 128], ident_b)
                qs = slice(q * 512, (q + 1) * 512)
                for h in range(2):
                    nc.vector.tensor_add(obs[h][:, qs], pts[h], xbs[h][:, qs])

            for h in range(2):
                nc.sync.dma_start(out=out_d[h * 128:(h + 1) * 128, g, :],
                                  in_=obs[h])

    _split_wide_waits(nc)
    return nc


_NC_CACHE = None


def _get_nc():
    global _NC_CACHE
    if _NC_CACHE is None:
        _NC_CACHE = build_nc()
    return _NC_CACHE


def _prep_inputs(x, Wq, bq, Wk, bk, Wv, bv, gamma):
    """Host-side: pad + window-major permute + shard x; pack weights."""
    xp = np.zeros((B, C, PH, PW), np.float32)
    xp[:, :, :H, :W] = x
    # window-major: [b, c, nh, nw, r, wc]
    xw = xp.reshape(B, C, NH, A, NW, A).transpose(0, 1, 2, 4, 3, 5)
    xw = np.ascontiguousarray(xw).reshape(B, C, NH, PIX)
    xw = xw.astype(ml_dtypes.bfloat16)
    # shard: core (2b + hrow) gets wrows [hrow*16, (hrow+1)*16) of batch b
    shards = []
    for core in range(NCORES):
        b, hr = core // 2, core % 2
        shards.append(np.ascontiguousarray(xw[b, :, hr * G:(hr + 1) * G, :]))

    wqk = np.concatenate([Wq.T, Wk.T], axis=1)          # [256, 64]
    wqk = wqk.reshape(2, 128, 64).astype(ml_dtypes.bfloat16)
    wvt = Wv.T.reshape(2, 128, 256).astype(ml_dtypes.bfloat16)  # [in, out]
    bqk = np.concatenate([bq, bk]).reshape(1, 64).astype(ml_dtypes.bfloat16)
    bvf = bv.astype(np.float32)
    ig = (1.0 / gamma.astype(np.float64)).astype(ml_dtypes.bfloat16).reshape(1)
    ident = np.eye(128, dtype=ml_dtypes.bfloat16)

    in_maps = []
    for core in range(NCORES):
        in_maps.append({
            "x": shards[core],
            "wqk": wqk,
            "wvt": wvt,
            "bqk": bqk,
            "bv": bvf,
            "igamma": ig,
            "ident": ident,
        })
    return in_maps


def _gather_output(results):
    full = np.stack([results[i]["out"] for i in range(NCORES)])  # [8,256,16,2048]
    full = full.reshape(B, 2, C, G, PIX).transpose(0, 2, 1, 3, 4)
    full = full.reshape(B, C, NH, NW, A, A).transpose(0, 1, 2, 4, 3, 5)
    full = np.ascontiguousarray(full).reshape(B, C, PH, PW)
    return np.ascontiguousarray(full[:, :, :H, :W])


def run(inputs, trace=False):
    nc = _get_nc()
    in_maps = _prep_inputs(**inputs)
    res = run_bass_kernel_spmd(nc, in_maps, core_ids=list(range(NCORES)),
                               trace=trace)
    return _gather_output(res.results), res


def kernel(**inputs):
    out, _ = run(inputs)
    return out
